# revision 1
# baseline (speedup 1.0000x reference)
"""CapsLayer2D dynamic-routing kernel for 8 Trainium2 NeuronCores.

Full inputs:  inputs [32,14,14,32,8] f32, W [16,32,8,16] f32
Full output:  out [32,14,14,16,16] f32

Sharding: pure data parallel over batch (4 batches / core -> 784 routing
locations per core). W replicated (rearranged host-side into a dense
[256,256] "sum" matrix and a block-diagonal [256,8192] matrix).
"""

import sys

sys.path.insert(0, "/opt/trn_rl_repo")

import numpy as np

import concourse.bass as bass
import concourse.mybir as mybir
from concourse.bacc import Bacc
from concourse.tile import TileContext

F32 = mybir.dt.float32
ADD = mybir.AluOpType.add
MULT = mybir.AluOpType.mult
AX = mybir.AxisListType.X
EXP = mybir.ActivationFunctionType.Exp
SQRT = mybir.ActivationFunctionType.Sqrt

EPS = 1e-7
B, R, C, N, I = 32, 14, 14, 32, 8
K, J = 16, 16
NCORES = 8
BC = B // NCORES            # batches per core
L = BC * R * C              # 784 locations per core
PT = 112                    # locations per partition-tile
NT = L // PT                # 7 tiles
NI = N * I                  # 256
KJ = K * J                  # 256
KN = K * N                  # 512
KNJ = K * N * J             # 8192


def _ap(base, dims):
    """AP over tile `base` ([part, free] contiguous) with free dims
    [(step,count)...] in elements; step 0 = broadcast."""
    return bass.AP(base.tensor, base.offset, [list(base.ap[0])] + [list(d) for d in dims])


def build_bass():
    nc = Bacc()
    x_d = nc.declare_dram_parameter("x", [L, NI], F32, isOutput=False)
    wsum_d = nc.declare_dram_parameter("wsum", [2, 128, KJ], F32, isOutput=False)
    wbd_d = nc.declare_dram_parameter("wbd", [2, 128, KNJ // 2], F32, isOutput=False)
    eye_d = nc.declare_dram_parameter("eye", [128, 128], F32, isOutput=False)
    out_d = nc.declare_dram_parameter("out", [L, KJ], F32, isOutput=True)

    with TileContext(nc) as tc:
        import contextlib
        ctx = contextlib.ExitStack()
        with ctx:
            cpool = ctx.enter_context(tc.tile_pool(name="const", bufs=1))
            wpool = ctx.enter_context(tc.tile_pool(name="work", bufs=2))
            bigpool = ctx.enter_context(tc.tile_pool(name="big", bufs=2))
            tmppool = ctx.enter_context(tc.tile_pool(name="tmp", bufs=1))
            pspool = ctx.enter_context(tc.tile_pool(name="ps", bufs=2, space="PSUM"))
            psmm = ctx.enter_context(tc.tile_pool(name="psmm", bufs=3, space="PSUM"))

            wsum0 = cpool.tile([128, KJ], F32)
            wsum1 = cpool.tile([128, KJ], F32)
            wbd0 = cpool.tile([128, KNJ // 2], F32)
            wbd1 = cpool.tile([128, KNJ // 2], F32)
            eye = cpool.tile([128, 128], F32)
            nc.gpsimd.dma_start(wsum0[:], wsum_d[0])
            nc.gpsimd.dma_start(wsum1[:], wsum_d[1])
            nc.gpsimd.dma_start(wbd0[:], wbd_d[0])
            nc.gpsimd.dma_start(wbd1[:], wbd_d[1])
            nc.gpsimd.dma_start(eye[:], eye_d[:])

            # PE warm-up: absorb the eye/wsum/wbd DMA ticks into PE's vector
            # clock one at a time, so no later LDWEIGHTS needs >1 sync wait
            # (HW limit: one wait slot on LDW).
            ps_w = pspool.tile([128, 512], F32, tag="psT", name="ps_w")
            nc.tensor.transpose(ps_w[:, :128], eye[:], eye[:])
            for wt in (wsum0, wsum1):
                nc.tensor.matmul(ps_w[:, :KJ], wt[:, :128], wt[:],
                                 start=True, stop=True)
            for wt in (wbd0, wbd1):
                nc.tensor.matmul(ps_w[:], wt[:, :128], wt[:, :512],
                                 start=True, stop=True)

            def squash(s_sb, out_sb, tag):
                """out = squash(s) over j; s_sb/out_sb [PT, KJ] f32 (k-major)."""
                tmp_s = wpool.tile([PT, KJ], F32, tag=f"sqt{tag}", name=f"sqt{tag}")
                sq = wpool.tile([PT, K], F32, tag=f"sq{tag}", name=f"sq{tag}")
                den = wpool.tile([PT, K], F32, tag=f"den{tag}", name=f"den{tag}")
                rden = wpool.tile([PT, K], F32, tag=f"rden{tag}", name=f"rden{tag}")
                rt = wpool.tile([PT, K], F32, tag=f"rt{tag}", name=f"rt{tag}")
                rti = wpool.tile([PT, K], F32, tag=f"rti{tag}", name=f"rti{tag}")
                rrt = wpool.tile([PT, K], F32, tag=f"rrt{tag}", name=f"rrt{tag}")
                f = wpool.tile([PT, K], F32, tag=f"f{tag}", name=f"f{tag}")
                nc.vector.tensor_tensor(tmp_s[:], s_sb[:], s_sb[:], MULT)
                nc.vector.tensor_reduce(
                    sq[:], _ap(tmp_s, [[J, K], [1, J]]), AX, ADD)
                nc.scalar.add(den[:], sq[:], 1.0)
                nc.vector.reciprocal(rden[:], den[:])
                nc.vector.tensor_scalar(rti[:], sq[:], EPS, None, ADD)
                nc.scalar.activation(rt[:], rti[:], SQRT)
                nc.vector.reciprocal(rrt[:], rt[:])
                nc.vector.tensor_tensor(f[:], sq[:], rden[:], MULT)
                nc.vector.tensor_tensor(f[:], f[:], rrt[:], MULT)
                nc.vector.tensor_tensor(
                    _ap(out_sb, [[J, K], [1, J]]),
                    _ap(s_sb, [[J, K], [1, J]]),
                    _ap(f, [[1, K], [0, J]]),
                    MULT)

            for t in range(NT):
                x_sb = wpool.tile([PT, NI], F32, tag="x", name="x_sb", bufs=NT)
                nc.gpsimd.dma_start(x_sb[:], x_d[t * PT:(t + 1) * PT, :])

                # transpose x -> xT halves [128, PT]
                xt = []
                for h in range(2):
                    ps_t = pspool.tile([128, PT], F32, tag="psT", name="ps_t")
                    xth = wpool.tile([128, PT], F32, tag=f"xT{h}", name=f"xT{h}")
                    nc.tensor.transpose(
                        ps_t[:], x_sb[:, h * 128:(h + 1) * 128], eye[:PT, :PT])
                    nc.scalar.copy(xth[:], ps_t[:])
                    xt.append(xth)

                # predicted p2 [PT, (k n j)] via block-diag W; chunk ch = n-pair
                p2 = bigpool.tile([PT, KNJ], F32, tag="p2", name="p2")
                for ch in range(16):
                    h = ch // 8
                    wb = (wbd0, wbd1)[h]
                    ps = psmm.tile([PT, 512], F32, tag="mm", name="ps_mm")
                    nc.tensor.matmul(
                        ps[:], xt[h][:], wb[:, (ch % 8) * 512:(ch % 8 + 1) * 512],
                        start=True, stop=True)
                    # psum cols (d,k,j) -> p2 cols k*512 + (2ch+d)*16 + j
                    dst = bass.AP(p2.tensor, p2.offset + 2 * ch * J,
                                  [list(p2.ap[0]), [J, 2], [KN, K], [1, J]])
                    src = _ap(ps, [[KJ, 2], [J, K], [1, J]])
                    eng = nc.scalar if ch % 2 else nc.vector
                    if ch % 2:
                        eng.copy(dst, src)
                    else:
                        eng.tensor_copy(dst, src)

                # iteration 1: c uniform -> s = (x @ wsum)/32
                ps_s = pspool.tile([PT, KJ], F32, tag="s", name="ps_s")
                nc.tensor.matmul(ps_s[:], xt[0][:], wsum0[:], start=True, stop=False)
                nc.tensor.matmul(ps_s[:], xt[1][:], wsum1[:], start=False, stop=True)
                s_sb = wpool.tile([PT, KJ], F32, tag="s_sb", name="s_sb")
                nc.scalar.mul(s_sb[:], ps_s[:], 1.0 / N)
                out_sb = wpool.tile([PT, KJ], F32, tag="out0", name="out_sb")
                squash(s_sb, out_sb, "a")

                b_sb = wpool.tile([PT, KN], F32, tag="b", name="b_sb")
                for it in range(2):
                    # agreement: bn[l,k,n] = sum_j p2[l,k,n,j] * out[l,k,j]
                    tmp = tmppool.tile([PT, KNJ], F32, tag="tmp", name="tmp")
                    nc.vector.tensor_tensor(
                        tmp[:],
                        p2[:],
                        _ap(out_sb, [[J, K], [0, N], [1, J]]),
                        MULT)
                    if it == 0:
                        nc.vector.tensor_reduce(
                            b_sb[:], _ap(tmp, [[J, KN], [1, J]]), AX, ADD)
                    else:
                        bn = wpool.tile([PT, KN], F32, tag="bn", name="bn")
                        nc.vector.tensor_reduce(
                            bn[:], _ap(tmp, [[J, KN], [1, J]]), AX, ADD)
                        nc.vector.tensor_tensor(b_sb[:], b_sb[:], bn[:], ADD)
                    # softmax over n (unnormalized; b bounded, no max-sub)
                    e_sb = wpool.tile([PT, KN], F32, tag="e", name="e_sb")
                    nc.scalar.activation(e_sb[:], b_sb[:], EXP)
                    se = wpool.tile([PT, K], F32, tag="se", name="se")
                    nc.vector.tensor_reduce(
                        se[:], _ap(e_sb, [[N, K], [1, N]]), AX, ADD)
                    r = wpool.tile([PT, K], F32, tag="r", name="r")
                    nc.vector.reciprocal(r[:], se[:])
                    # ws[l,k,j] = sum_n e[l,k,n]*p2[l,k,n,j]; write (k j n) scatter
                    tmp2 = tmppool.tile([PT, KNJ], F32, tag="tmp2", name="tmp2")
                    nc.vector.tensor_tensor(
                        _ap(tmp2, [[KN, K], [1, N], [N, J]]),
                        _ap(p2, [[KN, K], [J, N], [1, J]]),
                        _ap(e_sb, [[N, K], [1, N], [0, J]]),
                        MULT)
                    ws = wpool.tile([PT, KJ], F32, tag="ws", name="ws")
                    nc.vector.tensor_reduce(
                        ws[:], _ap(tmp2, [[N, KJ], [1, N]]), AX, ADD)
                    # s = ws * r (fold softmax normalizer), then squash
                    s2 = wpool.tile([PT, KJ], F32, tag="s2", name="s2")
                    nc.vector.tensor_tensor(
                        _ap(s2, [[J, K], [1, J]]),
                        _ap(ws, [[J, K], [1, J]]),
                        _ap(r, [[1, K], [0, J]]),
                        MULT)
                    out_sb = wpool.tile([PT, KJ], F32, tag=f"out{it + 1}",
                                        name="out_it")
                    squash(s2, out_sb, f"i{it}")

                nc.gpsimd.dma_start(out_d[t * PT:(t + 1) * PT, :], out_sb[:])
    nc.compile()
    return nc


def host_prep(inputs, W):
    x = np.ascontiguousarray(inputs, np.float32).reshape(NCORES, L, NI)
    wsum = np.ascontiguousarray(
        W.transpose(1, 2, 0, 3).reshape(NI, KJ), np.float32).reshape(2, 128, KJ)
    wbd_full = np.zeros((NI, KNJ), np.float32)
    for n in range(N):
        wbd_full[n * I:(n + 1) * I, n * KJ:(n + 1) * KJ] = (
            W[:, n].transpose(1, 0, 2).reshape(I, KJ))
    wbd = np.stack([wbd_full[0:128, 0:KNJ // 2],
                    wbd_full[128:256, KNJ // 2:]])
    eye = np.eye(128, dtype=np.float32)
    return x, wsum, wbd, eye


_CACHED = {}


def kernel(inputs, W):
    from concourse.bass_utils import run_bass_kernel_spmd

    x, wsum, wbd, eye = host_prep(inputs, W)
    if "nc" not in _CACHED:
        _CACHED["nc"] = build_bass()
    nc = _CACHED["nc"]
    in_maps = [{"x": np.ascontiguousarray(x[c]), "wsum": wsum, "wbd": wbd,
                "eye": eye} for c in range(NCORES)]
    res = run_bass_kernel_spmd(nc, in_maps, core_ids=list(range(NCORES)))
    out = np.stack([res.results[c]["out"] for c in range(NCORES)])
    return out.reshape(B, R, C, K, J)

